# revision 21
# baseline (speedup 1.0000x reference)
"""Causal self-attention with RoPE, tensor-parallel over 8 NeuronCores.

Problem shapes: x [2, 2048, 2048], w_qkv [6144, 2048], w_out [2048, 2048],
H=16 heads, dh=128. Core c owns heads {2c, 2c+1}.

Per-core plan (v3 — tuned against HW phase timings):
  - host passes xT [B, D, L] plus per-core transposed weight shards
  - 256-token chunks: Q^T/K^T psum[dh, 256] = W^T-tile (stationary) x xT
    (moving); V is projected into natural [tok, dh] layout (x token
    slices stationary, wvT moving) and copied back as bf16
  - RoPE during PSUM->SBUF copyback in 4 DVE ops (was 6): crossed-base
    t = swap(ps)*sin_signed (sin table carries the - sign for the first
    half), a = ps*cos_full (cos rows duplicated), dst = a + t
  - attention per 512-wide q chunk, kt-loop outer / head inner, AV+sum
    matmuls deferred one kt so the ACT exp (e in bf16) overlaps PE work;
    causal mask = bf16 multiply on the 4 diagonal-block tiles only
  - softmax denominator via ones^T x E matmuls; reciprocal + gpsimd
    partition_broadcast + multiply on the copyback (attn out in bf16)
  - PSUM: psA x4 holds QKV projections + attention score tiles + the two
    per-head attention-out accumulators; psB x2 holds V projections and
    the denominator rows; psW x2 holds w_out partials
  - w_out (bf16) partial[tok, e] accumulated over the 2 heads in PSUM;
    the 16 output tiles per q-chunk are drip-fed between attention kt
    cycles / QKV chunks, copied back on DVE (ACT is busy with exp), out
    DMA'd as bf16 on the scalar/gpsimd queues; host sums partials in f32
"""

import numpy as np
import ml_dtypes

import concourse.bass as bass
import concourse.mybir as mybir
import concourse.tile as tile
from concourse import bacc, library_config
from concourse.bass_utils import run_bass_kernel_spmd

B, L, D, H = 2, 2048, 2048, 16
DH = D // H  # 128
NCORES = 8
HPC = H // NCORES  # heads per core
ROPE_BASE = 10000.0
SCALE = 1.0 / float(np.sqrt(np.float32(DH)))

TOKC = 512  # token chunk width in the QKV projection phase
NCHUNK = L // TOKC  # 4
QC = 512  # q chunk width in the attention phase
NQC = L // QC  # 4
KT = L // 128  # 16 k tiles per sequence
KD = D // 128  # 16 contraction chunks for the projections

F32 = mybir.dt.float32
F32R = mybir.dt.float32r
BF16 = mybir.dt.bfloat16
AF = mybir.ActivationFunctionType
ALU = mybir.AluOpType


def _body(nc, tc, aps, phases=("qkv", "attn", "wout")):
    xt, wq, wk, wv, wo, cosf, sins, mk, out = aps
    with (
        tc.tile_pool(name="const", bufs=1) as const,
        tc.tile_pool(name="xtp", bufs=2) as xtp,
        tc.tile_pool(name="qkv", bufs=2) as qkvp,
        tc.tile_pool(name="tmps", bufs=2) as tmps,
        tc.tile_pool(name="esb", bufs=4) as esbp,
        tc.tile_pool(name="esum", bufs=3) as esump,
        tc.tile_pool(name="bcp", bufs=2) as bcp,
        tc.tile_pool(name="attn", bufs=4) as attnp,
        tc.tile_pool(name="outp", bufs=6) as outp,
        tc.tile_pool(name="psA", bufs=4, space="PSUM") as psA,
        tc.tile_pool(name="psB", bufs=2, space="PSUM") as psB,
        tc.tile_pool(name="psW", bufs=2, space="PSUM") as psW,
    ):
        # ---- constants ----
        wq_sb = const.tile([128, KD, HPC * DH], BF16, name="wq_sb")
        wk_sb = const.tile([128, KD, HPC * DH], BF16, name="wk_sb")
        wv_sb = const.tile([128, KD, HPC * DH], BF16, name="wv_sb")
        wo_sb = const.tile([128, HPC, D], BF16, name="wo_sb")
        cosf_sb = const.tile([128, L], F32, name="cosf_sb")
        sins_sb = const.tile([128, L], F32, name="sins_sb")
        mk_sb = const.tile([128, 4, QC], BF16, name="mk_sb")

        def load_chunk(b, c):
            c0 = c * TOKC
            xtile = xtp.tile([128, KD, TOKC], BF16, name="xtile")
            src = xt[b, :, c0:c0 + TOKC].rearrange("(ko p) n -> p ko n", p=128)
            # two half-loads: matmuls on k<8 start as soon as half 0 lands
            nc.sync.dma_start(xtile[:, 0:KD // 2], src[:, 0:KD // 2])
            nc.sync.dma_start(xtile[:, KD // 2:], src[:, KD // 2:])
            return xtile

        # first x chunk + wq ahead of everything else so the first QKV
        # matmul starts early; wo/mask last (first needed much later)
        xtile00 = load_chunk(0, 0) if "qkv" in phases else None
        for dst, src in ((wq_sb, wq), (cosf_sb, cosf), (sins_sb, sins),
                         (wk_sb, wk), (wv_sb, wv), (wo_sb, wo), (mk_sb, mk)):
            nc.sync.dma_start(dst, src)
        # all-ones stationary: the denominator matmul ones_sq^T x E writes
        # the column sums to ALL 128 psum partitions (a [1,N] matmul output
        # drains ~4x slower, and this doubles as the partition broadcast)
        ones_f32 = const.tile([128, 128], F32, name="ones_f32")
        nc.vector.memset(ones_f32, 1.0)
        ones_sq = const.tile([128, 128], BF16, name="ones_sq")
        nc.vector.tensor_copy(ones_sq, ones_f32)
        conste = None
        if "conste" in phases:
            ce_f32 = const.tile([128, QC], F32, name="ce_f32")
            nc.vector.memset(ce_f32, 0.001)
            conste = const.tile([128, QC], BF16, name="conste")
            nc.vector.tensor_copy(conste, ce_f32)

        # pending w_out tile emitters, drip-fed into later PE work
        wout_pend = []

        def drain_wout(n):
            for _ in range(min(n, len(wout_pend))):
                wout_pend.pop(0)()

        def queue_wout(attn_sb, b, q0):
            if "wout" not in phases:
                return
            for mt in range(QC // 128):
                t0 = q0 + mt * 128
                for ec in range(D // 512):
                    def piece(mt=mt, ec=ec, t0=t0, attn_sb=attn_sb, b=b):
                        psw = psW.tile([128, 512], F32, name="ps_w", tag="psW")
                        for h in range(HPC):
                            nc.tensor.matmul(
                                psw, attn_sb[h][:, mt * 128:(mt + 1) * 128],
                                wo_sb[:, h, ec * 512:(ec + 1) * 512],
                                start=(h == 0), stop=(h == HPC - 1),
                            )
                        ob = outp.tile([128, 512], BF16, name="out_sb")
                        if (mt + ec) % 2 == 0:
                            nc.scalar.copy(ob, psw)
                            nc.scalar.dma_start(
                                out[b, t0:t0 + 128, ec * 512:(ec + 1) * 512], ob)
                        else:
                            nc.vector.tensor_copy(ob, psw)
                            nc.gpsimd.dma_start(
                                out[b, t0:t0 + 128, ec * 512:(ec + 1) * 512], ob)
                    wout_pend.append(piece)

        for b in range(B):
            # ---- QKV projection + RoPE for batch b ----
            qrot = [qkvp.tile([128, L], BF16, name=f"qrot{h}") for h in range(HPC)]
            krot = [qkvp.tile([128, L], BF16, name=f"krot{h}") for h in range(HPC)]
            # both heads interleaved: AV stationary = vnat[:, kt, h*DH:(h+1)*DH]
            vnat = qkvp.tile([128, KT, HPC * DH], BF16, name="vnat")
            for c in range(NCHUNK if "qkv" in phases else 0):
                c0 = c * TOKC
                xtile = xtile00 if (b == 0 and c == 0) else load_chunk(b, c)
                for w_sb, dsts in ((wq_sb, qrot), (wk_sb, krot)):
                    for h in range(HPC):
                        ps = psA.tile([128, TOKC], F32, name="ps_qk", tag="psA")
                        for k in range(KD):
                            nc.tensor.matmul(
                                ps, w_sb[:, k, h * DH:(h + 1) * DH],
                                xtile[:, k, :],
                                start=(k == 0), stop=(k == KD - 1),
                            )
                        # RoPE copyback: dst = ps*cos + swap(ps)*sin_signed
                        cseg = cosf_sb[:, c0:c0 + TOKC]
                        sseg = sins_sb[:, c0:c0 + TOKC]
                        t = tmps.tile([128, TOKC], F32, name="rope_t")
                        a = tmps.tile([128, TOKC], F32, name="rope_a")
                        nc.vector.tensor_tensor(
                            t[0:64], ps[64:128], sseg[0:64], ALU.mult)
                        nc.vector.tensor_tensor(
                            t[64:128], ps[0:64], sseg[64:128], ALU.mult)
                        nc.vector.tensor_tensor(a, ps, cseg, ALU.mult)
                        nc.vector.tensor_tensor(
                            dsts[h][:, c0:c0 + TOKC], a, t, ALU.add)
                # V in natural [tok, dh] layout: x token slices stationary,
                # both heads share one moving pass; bf16 on copyback
                for s in range(TOKC // 128):
                    psv = psB.tile([128, HPC * DH], F32, name="ps_v", tag="psB")
                    for k in range(KD):
                        nc.tensor.matmul(
                            psv, xtile[:, k, s * 128:(s + 1) * 128], wv_sb[:, k, :],
                            start=(k == 0), stop=(k == KD - 1),
                        )
                    nc.vector.tensor_copy(vnat[:, (c0 // 128) + s, :], psv)
                drain_wout(2)

            # ---- attention per 512-token q chunk ----
            # head-outer; AV/sum matmuls deferred two kt steps so the ACT
            # exp roundtrip (sem + queue + 570ns) has a 2-cycle budget and
            # the score psum rotation stays 3 slots deep.
            for qc in range(NQC if "attn" in phases else 0):
                q0 = qc * QC
                nkt = (qc + 1) * (QC // 128)
                attn_sb = []
                for h in range(HPC):
                    pso = psA.tile([128, QC], F32, name="ps_out", tag="psA")
                    pss = psB.tile([128, QC], F32, name="ps_sum", tag="psB")
                    es = {}
                    eps = {}

                    def flush(kt):
                        src = es[kt] if "conste" not in phases else conste
                        vst = (vnat[:, kt, h * DH:(h + 1) * DH]
                               if "constv" not in phases else ones_sq)
                        nc.tensor.matmul(pso, vst,
                                         src, start=(kt == 0),
                                         stop=(kt == nkt - 1))
                        # denominator: one ones^T x (e_pair) matmul per kt
                        # pair; the pair pre-add runs on the DVE at 2x rate
                        if "nosum" not in phases and kt % 2 == 1:
                            p = kt // 2
                            nc.tensor.matmul(pss, ones_sq, eps.pop(p),
                                             start=(p == 0),
                                             stop=(p == nkt // 2 - 1))
                        es.pop(kt, None)

                    for kt in range(nkt):
                        psc = psA.tile([128, QC], F32, name="ps_sc", tag="psA")
                        kst = (krot[h][:, kt * 128:(kt + 1) * 128]
                               if "constk" not in phases else ones_sq)
                        nc.tensor.matmul(
                            psc, kst,
                            qrot[h][:, q0:q0 + QC], start=True, stop=True,
                        )
                        if "conste" not in phases:
                            e = esbp.tile([128, QC], BF16, name="e_sb")
                            if "noexp" in phases:
                                nc.scalar.copy(e, psc)
                            else:
                                nc.scalar.activation(e, psc, AF.Exp, scale=SCALE)
                            diag = kt - qc * (QC // 128)
                            if diag >= 0 and "nomask" not in phases:
                                nc.vector.tensor_tensor(
                                    e, e, mk_sb[:, diag, :], ALU.mult)
                            es[kt] = e
                            if "nosum" not in phases and kt % 2 == 1:
                                ep = esump.tile([128, QC], BF16, name="ep_sb")
                                nc.vector.tensor_tensor(
                                    ep, es[kt - 1], e, ALU.add)
                                eps[kt // 2] = ep
                        if kt >= 2:
                            flush(kt - 2)
                        drain_wout(2)
                    for kt in (nkt - 2, nkt - 1):
                        flush(kt)
                    # normalize: att = pso * (1/pss); pss already holds the
                    # sums on every partition, so no broadcast is needed
                    att = attnp.tile([128, QC], BF16, name="att")
                    if "nosum" in phases:
                        nc.vector.tensor_copy(att, pso)
                    else:
                        rec = bcp.tile([128, QC], F32, name="bc_sb")
                        nc.vector.reciprocal(rec, pss)
                        nc.vector.tensor_tensor(att, pso, rec, ALU.mult)
                    attn_sb.append(att)
                queue_wout(attn_sb, b, q0)
        drain_wout(len(wout_pend))


def build_kernel(timing=False, loop_n=0, phases=("qkv", "attn", "wout"),
                 trace_sim=False):
    nc = bacc.Bacc(
        "TRN2",
        target_bir_lowering=False,
        debug=False,
        enable_asserts=False,
        num_devices=NCORES,
    )
    xt = nc.dram_tensor("xt", [B, D, L], BF16, kind="ExternalInput").ap()
    wq = nc.dram_tensor("wq", [128, KD, HPC * DH], BF16, kind="ExternalInput").ap()
    wk = nc.dram_tensor("wk", [128, KD, HPC * DH], BF16, kind="ExternalInput").ap()
    wv = nc.dram_tensor("wv", [128, KD, HPC * DH], BF16, kind="ExternalInput").ap()
    wo = nc.dram_tensor("wo", [128, HPC, D], BF16, kind="ExternalInput").ap()
    cosf = nc.dram_tensor("cosf", [128, L], F32, kind="ExternalInput").ap()
    sins = nc.dram_tensor("sins", [128, L], F32, kind="ExternalInput").ap()
    mk = nc.dram_tensor("mk", [128, 4, QC], BF16, kind="ExternalInput").ap()
    out_kind = "Internal" if timing else "ExternalOutput"
    out = nc.dram_tensor("out", [B, L, D], BF16, kind=out_kind).ap()
    done = None
    if timing:
        done = nc.dram_tensor("done", [1, 4], BF16, kind="ExternalOutput").ap()

    nc.gpsimd.load_library(library_config.attn)
    aps = (xt, wq, wk, wv, wo, cosf, sins, mk, out)
    with tile.TileContext(nc, trace_sim=trace_sim) as tc:
        if loop_n:
            with tc.For_i(0, loop_n, 1):
                _body(nc, tc, aps, phases)
        else:
            _body(nc, tc, aps, phases)
        if timing:
            # tiny output so the executable has an ExternalOutput; depends on
            # one real out tile via a DRAM->DRAM DMA of the last row.
            nc.sync.dma_start(done, out[B - 1, L - 1:L, 0:4])
    nc.compile()
    return nc


def _rope_tables():
    inv_freq = (1.0 / (ROPE_BASE ** (np.arange(0, DH, 2, dtype=np.float32) / DH))
                ).astype(np.float32)
    freqs = (np.arange(L, dtype=np.float32)[:, None] * inv_freq[None, :]
             ).astype(np.float32)  # [L, 64]
    cos_t = np.cos(freqs).astype(np.float32).T  # [64, L]
    sin_t = np.sin(freqs).astype(np.float32).T
    cosf = np.concatenate([cos_t, cos_t], axis=0)  # [128, L]
    sins = np.concatenate([-sin_t, sin_t], axis=0)  # [128, L], sign folded in
    return np.ascontiguousarray(cosf), np.ascontiguousarray(sins)


def _host_inputs(x, w_qkv, w_out):
    xt = np.ascontiguousarray(np.transpose(x, (0, 2, 1))).astype(
        ml_dtypes.bfloat16)  # [B, D, L]
    cosf, sins = _rope_tables()
    p = np.arange(128)[:, None]
    f = np.arange(QC)[None, :]
    mk = np.stack(
        [((bi * 128 + p) <= f).astype(ml_dtypes.bfloat16) for bi in range(4)],
        axis=1,
    )  # [128, 4, 512]
    mk = np.ascontiguousarray(mk)

    def wtile(wT):  # [D, M] -> [128, D//128, M]
        return np.ascontiguousarray(
            wT.reshape(KD, 128, wT.shape[1]).transpose(1, 0, 2)
        ).astype(ml_dtypes.bfloat16)

    in_maps = []
    for c in range(NCORES):
        r0 = c * HPC * DH
        r1 = r0 + HPC * DH
        wq_c = wtile(np.ascontiguousarray(w_qkv[r0:r1, :].T))
        wk_c = wtile(np.ascontiguousarray(w_qkv[D + r0:D + r1, :].T))
        wv_c = wtile(np.ascontiguousarray(w_qkv[2 * D + r0:2 * D + r1, :].T))
        wo_c = np.ascontiguousarray(
            w_out[:, r0:r1].T.reshape(HPC, 128, D).transpose(1, 0, 2)
        ).astype(ml_dtypes.bfloat16)
        in_maps.append({
            "xt": xt, "wq": wq_c, "wk": wk_c, "wv": wv_c, "wo": wo_c,
            "cosf": cosf, "sins": sins, "mk": mk,
        })
    return in_maps


_NC_CACHE = []


def _get_nc():
    if not _NC_CACHE:
        _NC_CACHE.append(build_kernel())
    return _NC_CACHE[0]


def kernel(x, w_qkv, w_out):
    x = np.asarray(x, dtype=np.float32)
    w_qkv = np.asarray(w_qkv, dtype=np.float32)
    w_out = np.asarray(w_out, dtype=np.float32)
    nc = _get_nc()
    in_maps = _host_inputs(x, w_qkv, w_out)
    res = run_bass_kernel_spmd(nc, in_maps, core_ids=list(range(NCORES)))
    acc = res.results[0]["out"].astype(np.float32)
    for c in range(1, NCORES):
        acc += res.results[c]["out"].astype(np.float32)
    return acc


# revision 26
# speedup vs baseline: 1.0118x; 1.0118x over previous
"""Causal self-attention with RoPE, tensor-parallel over 8 NeuronCores.

Problem shapes: x [2, 2048, 2048], w_qkv [6144, 2048], w_out [2048, 2048],
H=16 heads, dh=128. Core c owns heads {2c, 2c+1}.

Per-core plan (v3 — tuned against HW phase timings):
  - host passes xT [B, D, L] plus per-core transposed weight shards
  - 256-token chunks: Q^T/K^T psum[dh, 256] = W^T-tile (stationary) x xT
    (moving); V is projected into natural [tok, dh] layout (x token
    slices stationary, wvT moving) and copied back as bf16
  - RoPE during PSUM->SBUF copyback in 4 DVE ops (was 6): crossed-base
    t = swap(ps)*sin_signed (sin table carries the - sign for the first
    half), a = ps*cos_full (cos rows duplicated), dst = a + t
  - attention per 512-wide q chunk, kt-loop outer / head inner, AV+sum
    matmuls deferred one kt so the ACT exp (e in bf16) overlaps PE work;
    causal mask = bf16 multiply on the 4 diagonal-block tiles only
  - softmax denominator via ones^T x E matmuls; reciprocal + gpsimd
    partition_broadcast + multiply on the copyback (attn out in bf16)
  - PSUM: psA x4 holds QKV projections + attention score tiles + the two
    per-head attention-out accumulators; psB x2 holds V projections and
    the denominator rows; psW x2 holds w_out partials
  - w_out (bf16) partial[tok, e] accumulated over the 2 heads in PSUM;
    the 16 output tiles per q-chunk are drip-fed between attention kt
    cycles / QKV chunks, copied back on DVE (ACT is busy with exp), out
    DMA'd as bf16 on the scalar/gpsimd queues; host sums partials in f32
"""

import numpy as np
import ml_dtypes

import concourse.bass as bass
import concourse.mybir as mybir
import concourse.tile as tile
from concourse import bacc, library_config
from concourse.bass_utils import run_bass_kernel_spmd

B, L, D, H = 2, 2048, 2048, 16
DH = D // H  # 128
NCORES = 8
HPC = H // NCORES  # heads per core
ROPE_BASE = 10000.0
SCALE = 1.0 / float(np.sqrt(np.float32(DH)))

TOKC = 512  # token chunk width in the QKV projection phase
NCHUNK = L // TOKC  # 4
QC = 512  # q chunk width in the attention phase
NQC = L // QC  # 4
KT = L // 128  # 16 k tiles per sequence
KD = D // 128  # 16 contraction chunks for the projections

F32 = mybir.dt.float32
F32R = mybir.dt.float32r
BF16 = mybir.dt.bfloat16
AF = mybir.ActivationFunctionType
ALU = mybir.AluOpType


def _body(nc, tc, aps, phases=("qkv", "attn", "wout")):
    xt, wq, wk, wv, wo, cosf, sins, mk, out = aps
    with (
        tc.tile_pool(name="const", bufs=1) as const,
        tc.tile_pool(name="xtp", bufs=3) as xtp,
        tc.tile_pool(name="qkv", bufs=2) as qkvp,
        tc.tile_pool(name="tmps", bufs=2) as tmps,
        tc.tile_pool(name="esb", bufs=6) as esbp,
        tc.tile_pool(name="esum", bufs=4) as esump,
        tc.tile_pool(name="bcp", bufs=2) as bcp,
        tc.tile_pool(name="attn", bufs=4) as attnp,
        tc.tile_pool(name="outp", bufs=8) as outp,
        tc.tile_pool(name="psA", bufs=4, space="PSUM") as psA,
        tc.tile_pool(name="psB", bufs=2, space="PSUM") as psB,
        tc.tile_pool(name="psW", bufs=2, space="PSUM") as psW,
    ):
        # ---- constants ----
        wq_sb = const.tile([128, KD, HPC * DH], BF16, name="wq_sb")
        wk_sb = const.tile([128, KD, HPC * DH], BF16, name="wk_sb")
        wv_sb = const.tile([128, KD, HPC * DH], BF16, name="wv_sb")
        wo_sb = const.tile([128, HPC, D], BF16, name="wo_sb")
        cosf_sb = const.tile([128, L], F32, name="cosf_sb")
        sins_sb = const.tile([128, L], F32, name="sins_sb")
        mk_sb = const.tile([128, 4, QC], BF16, name="mk_sb")

        def load_chunk(b, c):
            c0 = c * TOKC
            xtile = xtp.tile([128, KD, TOKC], BF16, name="xtile")
            src = xt[b, :, c0:c0 + TOKC].rearrange("(ko p) n -> p ko n", p=128)
            # two half-loads: matmuls on k<8 start as soon as half 0 lands
            nc.sync.dma_start(xtile[:, 0:KD // 2], src[:, 0:KD // 2])
            nc.sync.dma_start(xtile[:, KD // 2:], src[:, KD // 2:])
            return xtile

        # first x chunk + wq ahead of everything else so the first QKV
        # matmul starts early; wo/mask last (first needed much later)
        xtile00 = load_chunk(0, 0) if "qkv" in phases else None
        for dst, src in ((wq_sb, wq), (cosf_sb, cosf), (sins_sb, sins),
                         (wk_sb, wk), (wv_sb, wv), (wo_sb, wo), (mk_sb, mk)):
            nc.sync.dma_start(dst, src)
        # all-ones stationary: the denominator matmul ones_sq^T x E writes
        # the column sums to ALL 128 psum partitions (a [1,N] matmul output
        # drains ~4x slower, and this doubles as the partition broadcast)
        ones_f32 = const.tile([128, 128], F32, name="ones_f32")
        nc.vector.memset(ones_f32, 1.0)
        ones_sq = const.tile([128, 128], BF16, name="ones_sq")
        nc.vector.tensor_copy(ones_sq, ones_f32)
        conste = None
        if "conste" in phases:
            ce_f32 = const.tile([128, QC], F32, name="ce_f32")
            nc.vector.memset(ce_f32, 0.001)
            conste = const.tile([128, QC], BF16, name="conste")
            nc.vector.tensor_copy(conste, ce_f32)

        # pending w_out tile emitters, drip-fed into later PE work
        wout_pend = []

        def drain_wout(n):
            for _ in range(min(n, len(wout_pend))):
                wout_pend.pop(0)()

        def queue_wout(attn_sb, b, q0):
            if "wout" not in phases:
                return
            for mt in range(QC // 128):
                t0 = q0 + mt * 128
                for ec in range(D // 512):
                    def piece(mt=mt, ec=ec, t0=t0, attn_sb=attn_sb, b=b):
                        psw = psW.tile([128, 512], F32, name="ps_w", tag="psW")
                        for h in range(HPC):
                            nc.tensor.matmul(
                                psw, attn_sb[h][:, mt * 128:(mt + 1) * 128],
                                wo_sb[:, h, ec * 512:(ec + 1) * 512],
                                start=(h == 0), stop=(h == HPC - 1),
                            )
                        ob = outp.tile([128, 512], BF16, name="out_sb")
                        nc.vector.tensor_copy(ob, psw)
                        eng = nc.scalar if (mt + ec) % 2 == 0 else nc.gpsimd
                        eng.dma_start(
                            out[b, t0:t0 + 128, ec * 512:(ec + 1) * 512], ob)
                    wout_pend.append(piece)

        for b in range(B):
            # ---- QKV projection + RoPE for batch b ----
            qrot = [qkvp.tile([128, L], BF16, name=f"qrot{h}") for h in range(HPC)]
            krot = [qkvp.tile([128, L], BF16, name=f"krot{h}") for h in range(HPC)]
            # both heads interleaved: AV stationary = vnat[:, kt, h*DH:(h+1)*DH]
            vnat = qkvp.tile([128, KT, HPC * DH], BF16, name="vnat")
            for c in range(NCHUNK if "qkv" in phases else 0):
                c0 = c * TOKC
                xtile = xtile00 if (b == 0 and c == 0) else load_chunk(b, c)
                for w_sb, dsts in ((wq_sb, qrot), (wk_sb, krot)):
                    for h in range(HPC):
                        ps = psA.tile([128, TOKC], F32, name="ps_qk", tag="psA")
                        for k in range(KD):
                            nc.tensor.matmul(
                                ps, w_sb[:, k, h * DH:(h + 1) * DH],
                                xtile[:, k, :],
                                start=(k == 0), stop=(k == KD - 1),
                            )
                        # RoPE copyback: dst = ps*cos + swap(ps)*sin_signed
                        cseg = cosf_sb[:, c0:c0 + TOKC]
                        sseg = sins_sb[:, c0:c0 + TOKC]
                        t = tmps.tile([128, TOKC], F32, name="rope_t")
                        a = tmps.tile([128, TOKC], F32, name="rope_a")
                        nc.vector.tensor_tensor(
                            t[0:64], ps[64:128], sseg[0:64], ALU.mult)
                        nc.vector.tensor_tensor(
                            t[64:128], ps[0:64], sseg[64:128], ALU.mult)
                        nc.vector.tensor_tensor(a, ps, cseg, ALU.mult)
                        nc.vector.tensor_tensor(
                            dsts[h][:, c0:c0 + TOKC], a, t, ALU.add)
                # V in natural [tok, dh] layout: x token slices stationary,
                # both heads share one moving pass; bf16 on copyback
                for s in range(TOKC // 128):
                    psv = psB.tile([128, HPC * DH], F32, name="ps_v", tag="psB")
                    for k in range(KD):
                        nc.tensor.matmul(
                            psv, xtile[:, k, s * 128:(s + 1) * 128], wv_sb[:, k, :],
                            start=(k == 0), stop=(k == KD - 1),
                        )
                    nc.vector.tensor_copy(vnat[:, (c0 // 128) + s, :], psv)
                drain_wout(2)

            # ---- attention per 512-token q chunk ----
            # head-outer; AV/sum matmuls deferred two kt steps so the ACT
            # exp roundtrip (sem + queue + 570ns) has a 2-cycle budget and
            # the score psum rotation stays 3 slots deep.
            for qc in range(NQC if "attn" in phases else 0):
                q0 = qc * QC
                nkt = (qc + 1) * (QC // 128)
                attn_sb = []
                for h in range(HPC):
                    pso = psA.tile([128, QC], F32, name="ps_out", tag="psA")
                    pss = psB.tile([128, QC], F32, name="ps_sum", tag="psB")
                    es = {}
                    eps = {}

                    def flush(kt):
                        src = es[kt] if "conste" not in phases else conste
                        vst = (vnat[:, kt, h * DH:(h + 1) * DH]
                               if "constv" not in phases else ones_sq)
                        nc.tensor.matmul(pso, vst,
                                         src, start=(kt == 0),
                                         stop=(kt == nkt - 1))
                        # denominator: one ones^T x (e_pair) matmul per kt
                        # pair; the pair pre-add runs on the DVE at 2x rate
                        if "nosum" not in phases and kt % 2 == 1:
                            p = kt // 2
                            nc.tensor.matmul(pss, ones_sq, eps.pop(p),
                                             start=(p == 0),
                                             stop=(p == nkt // 2 - 1))
                        es.pop(kt, None)

                    for kt in range(nkt):
                        psc = psA.tile([128, QC], F32, name="ps_sc", tag="psA")
                        kst = (krot[h][:, kt * 128:(kt + 1) * 128]
                               if "constk" not in phases else ones_sq)
                        nc.tensor.matmul(
                            psc, kst,
                            qrot[h][:, q0:q0 + QC], start=True, stop=True,
                        )
                        if "conste" not in phases:
                            e = esbp.tile([128, QC], BF16, name="e_sb")
                            if "dvecopy" in phases:
                                nc.vector.tensor_copy(e, psc)
                            elif "noexp" in phases:
                                nc.scalar.copy(e, psc)
                            else:
                                nc.scalar.activation(e, psc, AF.Exp, scale=SCALE)
                            diag = kt - qc * (QC // 128)
                            if diag >= 0 and "nomask" not in phases:
                                nc.vector.tensor_tensor(
                                    e, e, mk_sb[:, diag, :], ALU.mult)
                            es[kt] = e
                            if "nosum" not in phases and kt % 2 == 1:
                                ep = esump.tile([128, QC], BF16, name="ep_sb")
                                nc.vector.tensor_tensor(
                                    ep, es[kt - 1], e, ALU.add)
                                eps[kt // 2] = ep
                        if kt >= 2:
                            flush(kt - 2)
                        drain_wout(2)
                    for kt in (nkt - 2, nkt - 1):
                        flush(kt)
                    # normalize: att = pso * (1/pss); pss already holds the
                    # sums on every partition, so no broadcast is needed
                    att = attnp.tile([128, QC], BF16, name="att")
                    if "nosum" in phases:
                        nc.vector.tensor_copy(att, pso)
                    else:
                        rec = bcp.tile([128, QC], F32, name="bc_sb")
                        nc.vector.reciprocal(rec, pss)
                        nc.vector.tensor_tensor(att, pso, rec, ALU.mult)
                    attn_sb.append(att)
                queue_wout(attn_sb, b, q0)
        drain_wout(len(wout_pend))


def build_kernel(timing=False, loop_n=0, phases=("qkv", "attn", "wout"),
                 trace_sim=False):
    nc = bacc.Bacc(
        "TRN2",
        target_bir_lowering=False,
        debug=False,
        enable_asserts=False,
        num_devices=NCORES,
    )
    xt = nc.dram_tensor("xt", [B, D, L], BF16, kind="ExternalInput").ap()
    wq = nc.dram_tensor("wq", [128, KD, HPC * DH], BF16, kind="ExternalInput").ap()
    wk = nc.dram_tensor("wk", [128, KD, HPC * DH], BF16, kind="ExternalInput").ap()
    wv = nc.dram_tensor("wv", [128, KD, HPC * DH], BF16, kind="ExternalInput").ap()
    wo = nc.dram_tensor("wo", [128, HPC, D], BF16, kind="ExternalInput").ap()
    cosf = nc.dram_tensor("cosf", [128, L], F32, kind="ExternalInput").ap()
    sins = nc.dram_tensor("sins", [128, L], F32, kind="ExternalInput").ap()
    mk = nc.dram_tensor("mk", [128, 4, QC], BF16, kind="ExternalInput").ap()
    out_kind = "Internal" if timing else "ExternalOutput"
    out = nc.dram_tensor("out", [B, L, D], BF16, kind=out_kind).ap()
    done = None
    if timing:
        done = nc.dram_tensor("done", [1, 4], BF16, kind="ExternalOutput").ap()

    nc.gpsimd.load_library(library_config.attn)
    aps = (xt, wq, wk, wv, wo, cosf, sins, mk, out)
    with tile.TileContext(nc, trace_sim=trace_sim) as tc:
        if loop_n:
            with tc.For_i(0, loop_n, 1):
                _body(nc, tc, aps, phases)
        else:
            _body(nc, tc, aps, phases)
        if timing:
            # tiny output so the executable has an ExternalOutput; depends on
            # one real out tile via a DRAM->DRAM DMA of the last row.
            nc.sync.dma_start(done, out[B - 1, L - 1:L, 0:4])
    nc.compile()
    return nc


def _rope_tables():
    inv_freq = (1.0 / (ROPE_BASE ** (np.arange(0, DH, 2, dtype=np.float32) / DH))
                ).astype(np.float32)
    freqs = (np.arange(L, dtype=np.float32)[:, None] * inv_freq[None, :]
             ).astype(np.float32)  # [L, 64]
    cos_t = np.cos(freqs).astype(np.float32).T  # [64, L]
    sin_t = np.sin(freqs).astype(np.float32).T
    cosf = np.concatenate([cos_t, cos_t], axis=0)  # [128, L]
    sins = np.concatenate([-sin_t, sin_t], axis=0)  # [128, L], sign folded in
    return np.ascontiguousarray(cosf), np.ascontiguousarray(sins)


def _host_inputs(x, w_qkv, w_out):
    xt = np.ascontiguousarray(np.transpose(x, (0, 2, 1))).astype(
        ml_dtypes.bfloat16)  # [B, D, L]
    cosf, sins = _rope_tables()
    p = np.arange(128)[:, None]
    f = np.arange(QC)[None, :]
    mk = np.stack(
        [((bi * 128 + p) <= f).astype(ml_dtypes.bfloat16) for bi in range(4)],
        axis=1,
    )  # [128, 4, 512]
    mk = np.ascontiguousarray(mk)

    def wtile(wT):  # [D, M] -> [128, D//128, M]
        return np.ascontiguousarray(
            wT.reshape(KD, 128, wT.shape[1]).transpose(1, 0, 2)
        ).astype(ml_dtypes.bfloat16)

    in_maps = []
    for c in range(NCORES):
        r0 = c * HPC * DH
        r1 = r0 + HPC * DH
        wq_c = wtile(np.ascontiguousarray(w_qkv[r0:r1, :].T))
        wk_c = wtile(np.ascontiguousarray(w_qkv[D + r0:D + r1, :].T))
        wv_c = wtile(np.ascontiguousarray(w_qkv[2 * D + r0:2 * D + r1, :].T))
        wo_c = np.ascontiguousarray(
            w_out[:, r0:r1].T.reshape(HPC, 128, D).transpose(1, 0, 2)
        ).astype(ml_dtypes.bfloat16)
        in_maps.append({
            "xt": xt, "wq": wq_c, "wk": wk_c, "wv": wv_c, "wo": wo_c,
            "cosf": cosf, "sins": sins, "mk": mk,
        })
    return in_maps


_NC_CACHE = []


def _get_nc():
    if not _NC_CACHE:
        _NC_CACHE.append(build_kernel())
    return _NC_CACHE[0]


def kernel(x, w_qkv, w_out):
    x = np.asarray(x, dtype=np.float32)
    w_qkv = np.asarray(w_qkv, dtype=np.float32)
    w_out = np.asarray(w_out, dtype=np.float32)
    nc = _get_nc()
    in_maps = _host_inputs(x, w_qkv, w_out)
    res = run_bass_kernel_spmd(nc, in_maps, core_ids=list(range(NCORES)))
    acc = res.results[0]["out"].astype(np.float32)
    for c in range(1, NCORES):
        acc += res.results[c]["out"].astype(np.float32)
    return acc


# revision 27
# speedup vs baseline: 1.0443x; 1.0322x over previous
"""Causal self-attention with RoPE on 8 NeuronCores, DP2 x TP4.

Problem shapes: x [2, 2048, 2048], w_qkv [6144, 2048], w_out [2048, 2048],
H=16 heads, dh=128. Core c owns batch c//4 and heads 4*(c%4)..4*(c%4)+3,
so each core runs ONE batch end-to-end (no cross-batch serialization)
and the host sums 4 partial outputs per batch.

Per-core plan (v10 — tuned against HW phase timings):
  - host passes xT [D, L] (own batch) plus per-core transposed bf16
    weight shards; everything the PE touches is bf16 except the psum
  - 512-token chunks: Q^T/K^T psum[dh, 512] = W^T-tile (stationary) x xT
    (moving); V psum[tok, 4*dh=512] = x token slices (stationary) x wvT
    (moving) — all projection matmuls are N=512
  - RoPE during PSUM->SBUF copyback in 4 DVE ops: crossed-base
    t = swap(ps)*sin_signed (sin table carries the - sign for the first
    half), a = ps*cos_full (cos rows duplicated), dst = a + t, written
    to PER-CHUNK qrot/krot tiles so attention q-chunk qc only depends on
    projection chunks <= qc (scheduler can overlap the phases)
  - attention per 512-wide q chunk, head-outer; AV matmuls deferred two
    kt steps so the ACT exp roundtrip (e in bf16) hides under PE work;
    causal mask = bf16 multiply on the 4 diagonal-block tiles
  - softmax denominator: e-pairs pre-added on DVE (bf16 2x), one
    all-ones [128,128] stationary matmul per pair accumulates the sums
    broadcast across all psum partitions (no gpsimd broadcast needed);
    normalize = reciprocal + multiply on the copyback
  - PSUM: psA x4 (QKV projections, score tiles, attention-out
    accumulator), psB x2 (V projections, denominator), psW x2 (w_out)
  - w_out partial[tok, e] accumulated over the 4 heads in PSUM; the 16
    output tiles per q-chunk are drip-fed between attention kt cycles,
    copied back on DVE, DMA'd as bf16 on the scalar/gpsimd queues; the
    host sums the 4 partials per batch in f32
"""

import numpy as np
import ml_dtypes

import concourse.bass as bass
import concourse.mybir as mybir
import concourse.tile as tile
from concourse import bacc, library_config
from concourse.bass_utils import run_bass_kernel_spmd

B, L, D, H = 2, 2048, 2048, 16
DH = D // H  # 128
NCORES = 8
DP = 2  # batch shards
TP = NCORES // DP  # 4 head-group shards
HPC = H // TP  # 4 heads per core
ROPE_BASE = 10000.0
SCALE = 1.0 / float(np.sqrt(np.float32(DH)))

TOKC = 512  # token chunk width in the QKV projection phase
NCHUNK = L // TOKC  # 4
QC = 512  # q chunk width in the attention phase
NQC = L // QC  # 4
KT = L // 128  # 16 k tiles per sequence
KD = D // 128  # 16 contraction chunks for the projections
KPC = TOKC // 128  # 4 k tiles per chunk

F32 = mybir.dt.float32
BF16 = mybir.dt.bfloat16
AF = mybir.ActivationFunctionType
ALU = mybir.AluOpType


def _body(nc, tc, aps, phases=("qkv", "attn", "wout")):
    xt, wq, wk, wv, wo, cosf, sins, mk, out = aps
    with (
        tc.tile_pool(name="const", bufs=1) as const,
        tc.tile_pool(name="xtp", bufs=2) as xtp,
        tc.tile_pool(name="qkv", bufs=1) as qkvp,
        tc.tile_pool(name="tmps", bufs=2) as tmps,
        tc.tile_pool(name="esb", bufs=5) as esbp,
        tc.tile_pool(name="esum", bufs=4) as esump,
        tc.tile_pool(name="bcp", bufs=2) as bcp,
        tc.tile_pool(name="attn", bufs=6) as attnp,
        tc.tile_pool(name="outp", bufs=6) as outp,
        tc.tile_pool(name="psA", bufs=4, space="PSUM") as psA,
        tc.tile_pool(name="psB", bufs=2, space="PSUM") as psB,
        tc.tile_pool(name="psW", bufs=2, space="PSUM") as psW,
    ):
        # ---- constants ----
        wq_sb = const.tile([128, KD, HPC * DH], BF16, name="wq_sb")
        wk_sb = const.tile([128, KD, HPC * DH], BF16, name="wk_sb")
        wv_sb = const.tile([128, KD, HPC * DH], BF16, name="wv_sb")
        wo_sb = const.tile([128, HPC, D], BF16, name="wo_sb")
        cosf_sb = const.tile([128, L], BF16, name="cosf_sb")
        sins_sb = const.tile([128, L], BF16, name="sins_sb")
        mk_sb = const.tile([128, 4, QC], BF16, name="mk_sb")

        def load_chunk(c):
            c0 = c * TOKC
            xtile = xtp.tile([128, KD, TOKC], BF16, name="xtile")
            src = xt[:, c0:c0 + TOKC].rearrange("(ko p) n -> p ko n", p=128)
            # two half-loads: matmuls on k<8 start as soon as half 0 lands
            nc.sync.dma_start(xtile[:, 0:KD // 2], src[:, 0:KD // 2])
            nc.sync.dma_start(xtile[:, KD // 2:], src[:, KD // 2:])
            return xtile

        # first x chunk + wq ahead of everything else so the first QKV
        # matmul starts early; wo/mask last (first needed much later)
        xtile0 = load_chunk(0) if "qkv" in phases else None
        for dst, src in ((wq_sb, wq), (cosf_sb, cosf), (sins_sb, sins),
                         (wk_sb, wk), (wv_sb, wv), (wo_sb, wo), (mk_sb, mk)):
            nc.sync.dma_start(dst, src)
        # all-ones stationary: the denominator matmul ones_sq^T x E writes
        # the column sums to ALL 128 psum partitions (a [1,N] matmul output
        # drains ~4x slower, and this doubles as the partition broadcast)
        ones_f32 = const.tile([128, 128], F32, name="ones_f32")
        nc.vector.memset(ones_f32, 1.0)
        ones_sq = const.tile([128, 128], BF16, name="ones_sq")
        nc.vector.tensor_copy(ones_sq, ones_f32)

        # pending w_out tile emitters, drip-fed into later PE work
        wout_pend = []

        def drain_wout(n):
            for _ in range(min(n, len(wout_pend))):
                wout_pend.pop(0)()

        def queue_wout(attn_sb, q0):
            if "wout" not in phases:
                return
            for mt in range(QC // 128):
                t0 = q0 + mt * 128
                for ec in range(D // 512):
                    def piece(mt=mt, ec=ec, t0=t0, attn_sb=attn_sb):
                        psw = psW.tile([128, 512], F32, name="ps_w", tag="psW")
                        for h in range(HPC):
                            nc.tensor.matmul(
                                psw, attn_sb[h][:, mt * 128:(mt + 1) * 128],
                                wo_sb[:, h, ec * 512:(ec + 1) * 512],
                                start=(h == 0), stop=(h == HPC - 1),
                            )
                        ob = outp.tile([128, 512], BF16, name="out_sb")
                        nc.vector.tensor_copy(ob, psw)
                        eng = nc.scalar if (mt + ec) % 2 == 0 else nc.gpsimd
                        eng.dma_start(
                            out[t0:t0 + 128, ec * 512:(ec + 1) * 512], ob)
                    wout_pend.append(piece)

        # ---- QKV projection + RoPE (single batch per core) ----
        # per-chunk tiles so attention q-chunk qc depends only on chunks<=qc
        qrot = [[qkvp.tile([128, TOKC], BF16, name=f"qrot{h}_{c}")
                 for c in range(NCHUNK)] for h in range(HPC)]
        krot = [[qkvp.tile([128, TOKC], BF16, name=f"krot{h}_{c}")
                 for c in range(NCHUNK)] for h in range(HPC)]
        vnat = [qkvp.tile([128, KPC, HPC * DH], BF16, name=f"vnat{c}")
                for c in range(NCHUNK)]
        for c in range(NCHUNK if "qkv" in phases else 0):
            c0 = c * TOKC
            xtile = xtile0 if c == 0 else load_chunk(c)
            for w_sb, dsts in ((wq_sb, qrot), (wk_sb, krot)):
                for h in range(HPC):
                    ps = psA.tile([128, TOKC], F32, name="ps_qk", tag="psA")
                    for k in range(KD):
                        nc.tensor.matmul(
                            ps, w_sb[:, k, h * DH:(h + 1) * DH],
                            xtile[:, k, :],
                            start=(k == 0), stop=(k == KD - 1),
                        )
                    # RoPE copyback: dst = ps*cos + swap(ps)*sin_signed
                    cseg = cosf_sb[:, c0:c0 + TOKC]
                    sseg = sins_sb[:, c0:c0 + TOKC]
                    t = tmps.tile([128, TOKC], F32, name="rope_t")
                    a = tmps.tile([128, TOKC], F32, name="rope_a")
                    nc.vector.tensor_tensor(
                        t[0:64], ps[64:128], sseg[0:64], ALU.mult)
                    nc.vector.tensor_tensor(
                        t[64:128], ps[0:64], sseg[64:128], ALU.mult)
                    nc.vector.tensor_tensor(a, ps, cseg, ALU.mult)
                    nc.vector.tensor_tensor(dsts[h][c], a, t, ALU.add)
            # V in natural [tok, dh] layout: x token slices stationary, all
            # four heads share one N=512 moving pass; bf16 on copyback
            for s in range(KPC):
                psv = psB.tile([128, HPC * DH], F32, name="ps_v", tag="psB")
                for k in range(KD):
                    nc.tensor.matmul(
                        psv, xtile[:, k, s * 128:(s + 1) * 128], wv_sb[:, k, :],
                        start=(k == 0), stop=(k == KD - 1),
                    )
                nc.vector.tensor_copy(vnat[c][:, s, :], psv)
            drain_wout(2)

        # ---- attention per 512-token q chunk ----
        # head-outer; AV matmuls deferred two kt steps so the ACT exp
        # roundtrip has a 2-cycle budget; denominator via e-pair matmuls
        for qc in range(NQC if "attn" in phases else 0):
            q0 = qc * QC
            nkt = (qc + 1) * KPC
            attn_sb = []
            for h in range(HPC):
                pso = psA.tile([128, QC], F32, name="ps_out", tag="psA")
                pss = psB.tile([128, QC], F32, name="ps_sum", tag="psB")
                es = {}
                eps = {}

                def flush(kt):
                    nc.tensor.matmul(
                        pso, vnat[kt // KPC][:, kt % KPC, h * DH:(h + 1) * DH],
                        es[kt], start=(kt == 0), stop=(kt == nkt - 1))
                    # denominator: one ones^T x (e_pair) matmul per kt pair;
                    # the pair pre-add runs on the DVE at 2x rate
                    if kt % 2 == 1:
                        p = kt // 2
                        nc.tensor.matmul(pss, ones_sq, eps.pop(p),
                                         start=(p == 0),
                                         stop=(p == nkt // 2 - 1))
                    es.pop(kt, None)

                for kt in range(nkt):
                    psc = psA.tile([128, QC], F32, name="ps_sc", tag="psA")
                    nc.tensor.matmul(
                        psc, krot[h][kt // KPC][:, (kt % KPC) * 128:
                                                (kt % KPC + 1) * 128],
                        qrot[h][qc], start=True, stop=True,
                    )
                    e = esbp.tile([128, QC], BF16, name="e_sb")
                    nc.scalar.activation(e, psc, AF.Exp, scale=SCALE)
                    diag = kt - qc * KPC
                    if diag >= 0:
                        nc.vector.tensor_tensor(
                            e, e, mk_sb[:, diag, :], ALU.mult)
                    es[kt] = e
                    if kt % 2 == 1:
                        ep = esump.tile([128, QC], BF16, name="ep_sb")
                        nc.vector.tensor_tensor(ep, es[kt - 1], e, ALU.add)
                        eps[kt // 2] = ep
                    if kt >= 2:
                        flush(kt - 2)
                    drain_wout(2)
                for kt in (nkt - 2, nkt - 1):
                    flush(kt)
                # normalize: att = pso * (1/pss); pss already holds the
                # sums on every partition, so no broadcast is needed
                att = attnp.tile([128, QC], BF16, name="att")
                rec = bcp.tile([128, QC], F32, name="bc_sb")
                nc.vector.reciprocal(rec, pss)
                nc.vector.tensor_tensor(att, pso, rec, ALU.mult)
                attn_sb.append(att)
            queue_wout(attn_sb, q0)
        drain_wout(len(wout_pend))


def build_kernel(timing=False, loop_n=0, phases=("qkv", "attn", "wout"),
                 trace_sim=False):
    nc = bacc.Bacc(
        "TRN2",
        target_bir_lowering=False,
        debug=False,
        enable_asserts=False,
        num_devices=NCORES,
    )
    xt = nc.dram_tensor("xt", [D, L], BF16, kind="ExternalInput").ap()
    wq = nc.dram_tensor("wq", [128, KD, HPC * DH], BF16, kind="ExternalInput").ap()
    wk = nc.dram_tensor("wk", [128, KD, HPC * DH], BF16, kind="ExternalInput").ap()
    wv = nc.dram_tensor("wv", [128, KD, HPC * DH], BF16, kind="ExternalInput").ap()
    wo = nc.dram_tensor("wo", [128, HPC, D], BF16, kind="ExternalInput").ap()
    cosf = nc.dram_tensor("cosf", [128, L], BF16, kind="ExternalInput").ap()
    sins = nc.dram_tensor("sins", [128, L], BF16, kind="ExternalInput").ap()
    mk = nc.dram_tensor("mk", [128, 4, QC], BF16, kind="ExternalInput").ap()
    out_kind = "Internal" if timing else "ExternalOutput"
    out = nc.dram_tensor("out", [L, D], BF16, kind=out_kind).ap()
    done = None
    if timing:
        done = nc.dram_tensor("done", [1, 4], BF16, kind="ExternalOutput").ap()

    nc.gpsimd.load_library(library_config.attn)
    aps = (xt, wq, wk, wv, wo, cosf, sins, mk, out)
    with tile.TileContext(nc, trace_sim=trace_sim) as tc:
        if loop_n:
            with tc.For_i(0, loop_n, 1):
                _body(nc, tc, aps, phases)
        else:
            _body(nc, tc, aps, phases)
        if timing:
            # tiny output so the executable has an ExternalOutput; depends on
            # one real out tile via a DRAM->DRAM DMA of the last row.
            nc.sync.dma_start(done, out[L - 1:L, 0:4])
    nc.compile()
    return nc


def _rope_tables():
    inv_freq = (1.0 / (ROPE_BASE ** (np.arange(0, DH, 2, dtype=np.float32) / DH))
                ).astype(np.float32)
    freqs = (np.arange(L, dtype=np.float32)[:, None] * inv_freq[None, :]
             ).astype(np.float32)  # [L, 64]
    cos_t = np.cos(freqs).astype(np.float32).T  # [64, L]
    sin_t = np.sin(freqs).astype(np.float32).T
    cosf = np.concatenate([cos_t, cos_t], axis=0)  # [128, L]
    sins = np.concatenate([-sin_t, sin_t], axis=0)  # [128, L], sign folded in
    return (np.ascontiguousarray(cosf).astype(ml_dtypes.bfloat16),
            np.ascontiguousarray(sins).astype(ml_dtypes.bfloat16))


def _host_inputs(x, w_qkv, w_out):
    cosf, sins = _rope_tables()
    p = np.arange(128)[:, None]
    f = np.arange(QC)[None, :]
    mk = np.stack(
        [((bi * 128 + p) <= f).astype(ml_dtypes.bfloat16) for bi in range(4)],
        axis=1,
    )  # [128, 4, 512]
    mk = np.ascontiguousarray(mk)

    def wtile(wT):  # [D, M] -> [128, D//128, M]
        return np.ascontiguousarray(
            wT.reshape(KD, 128, wT.shape[1]).transpose(1, 0, 2)
        ).astype(ml_dtypes.bfloat16)

    xts = [np.ascontiguousarray(x[b].T).astype(ml_dtypes.bfloat16)
           for b in range(B)]  # [D, L] per batch
    in_maps = []
    for c in range(NCORES):
        b = c // TP
        tp = c % TP
        r0 = tp * HPC * DH
        r1 = r0 + HPC * DH
        wq_c = wtile(np.ascontiguousarray(w_qkv[r0:r1, :].T))
        wk_c = wtile(np.ascontiguousarray(w_qkv[D + r0:D + r1, :].T))
        wv_c = wtile(np.ascontiguousarray(w_qkv[2 * D + r0:2 * D + r1, :].T))
        wo_c = np.ascontiguousarray(
            w_out[:, r0:r1].T.reshape(HPC, 128, D).transpose(1, 0, 2)
        ).astype(ml_dtypes.bfloat16)
        in_maps.append({
            "xt": xts[b], "wq": wq_c, "wk": wk_c, "wv": wv_c, "wo": wo_c,
            "cosf": cosf, "sins": sins, "mk": mk,
        })
    return in_maps


_NC_CACHE = []


def _get_nc():
    if not _NC_CACHE:
        _NC_CACHE.append(build_kernel())
    return _NC_CACHE[0]


def kernel(x, w_qkv, w_out):
    x = np.asarray(x, dtype=np.float32)
    w_qkv = np.asarray(w_qkv, dtype=np.float32)
    w_out = np.asarray(w_out, dtype=np.float32)
    nc = _get_nc()
    in_maps = _host_inputs(x, w_qkv, w_out)
    res = run_bass_kernel_spmd(nc, in_maps, core_ids=list(range(NCORES)))
    acc = np.zeros((B, L, D), dtype=np.float32)
    for c in range(NCORES):
        acc[c // TP] += res.results[c]["out"].astype(np.float32)
    return acc
